# revision 23
# baseline (speedup 1.0000x reference)
"""Trainium2 Bass kernel for nn_Attention_51634096833229.

Conv-projection attention block (CvT-style): depthwise 3x3 conv + BN on the
28x28 token image for each of q/k/v, linear qkv projections, 3-head attention
over 785 tokens (784 image + 1 cls), output projection.

Sharding: data-parallel over batch, B=32 -> 4 samples per core on 8 cores.

Per-core design (software-pipelined: sample b's conv/qkv emitted between
sample b-1's attention heads so the PE stream stays dense):
  x bf16 (host-converted, T padded to 800, C padded to 224)
    --XBAR DMA-transpose--> xT [192,800] bf16 (2 tiles, c 0:128 / 96:192)
  dw-conv+BN on the PE as fp8 diag matmuls: per-channel tap weights on the
  diagonal, 2 taps per DoubleRow matmul via a 3-plane shifted pad canvas
  [rowshift|canvas|colshift]; PSUM rows split (16,12) to stay in banks;
  BN bias folded into the psum->sbuf y copy (per-partition scalar)
  q,k: feature-major bf16 matmul -> qT,kT [192,785]
  v:   token-major bf16 matmul -> per-tblk vaug [t,3*65] with ones cols
  scores^T [t,l] = kT_h^T qT_h bf16; exp on ACT (scale folded, no max sub);
  PV: outT_h [65,l] accumulated over t; row 64 = softmax denominators;
  scores emitted 3 t-tiles ahead of PV to cover the exp latency
  normalize: ACT-staged den + DVE reciprocal_approx_fast + gpsimd
  partition-broadcast + DVE mult
  final: token-major matmul (lhsT = aT chunks, rhs = w_proj^T with ones-row
  bias) -> out bf16 --DMA--> DRAM, host converts to f32
"""

import sys

sys.path.insert(0, "/opt/trn_rl_repo")

import numpy as np
import ml_dtypes

import concourse.bass as bass
import concourse.mybir as mybir
import concourse.tile as tile
from concourse import bacc
from concourse.bass_utils import run_bass_kernel_spmd

F32 = mybir.dt.float32
BF16 = mybir.dt.bfloat16
FP8 = mybir.dt.float8e4
AF = mybir.ActivationFunctionType
OP = mybir.AluOpType

B, T, C, CO, NH, D = 32, 785, 192, 192, 3, 64
TP = 800  # host-padded token count (multiple of 16 for DMA transpose)
CP = 224  # host-padded channel count (so chunk1 transpose lands at rows 0-95)
HH = WW = 28
NCORES = 8
BPC = B // NCORES  # samples per core
SCALE = float(CO) ** -0.5
BN_EPS = 1e-5

# token blocks of 128 along T
TBLK = [(i * 128, min(128, T - i * 128)) for i in range((T + 127) // 128)]
# channel chunks along C=192
CCH = [(0, 96), (96, 96)]
# N segments within 785 (psum bank = 512 f32)
NSEG = [(0, 512), (512, T - 512)]
# conv psum row segments: (img row0, n rows, psum col offset)
ROWSEG = [(0, 16, 0), (16, 12, 512)]


def build_bass():
    nc = bacc.Bacc(None)
    x_d = nc.declare_dram_parameter("x", [BPC, TP, CP], BF16, isOutput=False)
    wqkvT_d = nc.declare_dram_parameter("wqkvT", [3, C, CO], BF16, isOutput=False)
    wdiag_d = nc.declare_dram_parameter("wdiag", [2, 96, 3 * 5 * 2 * 96], FP8,
                                        isOutput=False)
    bnt_d = nc.declare_dram_parameter("bnt", [C, 3], F32, isOutput=False)
    wpa_d = nc.declare_dram_parameter("wpa", [C + 1, CO], BF16, isOutput=False)
    out_d = nc.declare_dram_parameter("out", [BPC, T, CO], BF16, isOutput=True)

    from contextlib import ExitStack
    with tile.TileContext(nc) as tc, ExitStack() as es:
        consts = es.enter_context(tc.tile_pool(name="consts", bufs=1))
        psA = es.enter_context(tc.tile_pool(name="psA", bufs=4, space="PSUM"))
        xTp = es.enter_context(tc.tile_pool(name="xT", bufs=2))
        padp = es.enter_context(tc.tile_pool(name="pad", bufs=2))
        yp = es.enter_context(tc.tile_pool(name="y", bufs=2))
        qkp = es.enter_context(tc.tile_pool(name="qk", bufs=2))
        q2p = es.enter_context(tc.tile_pool(name="q2", bufs=2))
        vap = es.enter_context(tc.tile_pool(name="va", bufs=2))
        ep = es.enter_context(tc.tile_pool(name="E", bufs=6))
        atp = es.enter_context(tc.tile_pool(name="aT", bufs=2))
        op_ = es.enter_context(tc.tile_pool(name="osb", bufs=3))
        smallp = es.enter_context(tc.tile_pool(name="small", bufs=3))
        if True:
            wq_sb, wd_sb, bnt_sb, wpa_sb = [], [], [], []

            def emit_consts():
                # weights into SBUF, split by channel chunk (emitted after
                # sample 0's x DMAs so the first conv isn't queued behind them)
                for i in range(3):
                    row = []
                    for ci, (c0, cp) in enumerate(CCH):
                        t = consts.tile([cp, CO], BF16, tag=f"wq{i}{ci}",
                                        name=f"wq{i}{ci}")
                        nc.sync.dma_start(t[:], wqkvT_d[i, c0:c0 + cp, :])
                        row.append(t)
                    wq_sb.append(row)
                for ci, (c0, cp) in enumerate(CCH):
                    t = consts.tile([96, 3 * 5 * 2 * 96], FP8, tag=f"wd{ci}",
                                    name=f"wd{ci}")
                    nc.sync.dma_start(t[:], wdiag_d[ci])
                    wd_sb.append(t)
                    t2 = consts.tile([cp, 3], F32, tag=f"bnt{ci}", name=f"bnt{ci}")
                    nc.sync.dma_start(t2[:], bnt_d[c0:c0 + cp, :])
                    bnt_sb.append(t2)
                wpa0 = consts.tile([128, CO], BF16, tag="wpa0", name="wpa0")
                nc.sync.dma_start(wpa0[:], wpa_d[0:128, :])
                wpa1 = consts.tile([65, CO], BF16, tag="wpa1", name="wpa1")
                nc.sync.dma_start(wpa1[:], wpa_d[128:193, :])
                wpa_sb.extend([wpa0, wpa1])

            # parity-double-buffered persistent tiles (pipeline overlap)
            vaug = [[vap.tile([128, 3 * 65], BF16, tag=f"va{p_}{tb}",
                              name=f"va{p_}{tb}") for tb in range(len(TBLK))]
                    for p_ in range(2)]
            for p_ in range(2):
                for tb, (t0, tn) in enumerate(TBLK):
                    ones3 = vaug[p_][tb][:].rearrange(
                        "p (h d) -> p h d", h=3)[0:tn, :, 64:65]
                    nc.vector.memset(ones3, 1.0)
            aT0 = [atp.tile([128, T], BF16, tag=f"aT0{p_}", name=f"aT0{p_}")
                   for p_ in range(2)]
            aT1 = [atp.tile([65, T], BF16, tag=f"aT1{p_}", name=f"aT1{p_}")
                   for p_ in range(2)]
            for p_ in range(2):
                nc.vector.memset(aT1[p_][64:65, :], 1.0)

            def emit_load(b):
                """DMA-transposed x load + fp8 3-plane pad canvases."""
                xlo = xTp.tile([128, TP], BF16, tag="xlo", name="xlo")
                xhi = xTp.tile([128, TP], BF16, tag="xhi", name="xhi")
                nc.sync.dma_start(xlo[:], x_d[b, :, 0:128], transpose=True)
                nc.sync.dma_start(xhi[:], x_d[b, :, 96:224], transpose=True)
                cv = [xlo[0:96], xhi[0:96]]
                pads = []
                for ci in range(2):
                    pad = padp.tile([96, 2700], FP8, tag=f"pad{ci}",
                                    name=f"pad{ci}")
                    nc.vector.memset(pad[:, 900:1800], 0.0)
                    nc.vector.tensor_copy(
                        pad[:, 900:1800].rearrange(
                            "p (y x) -> p y x", y=30, x=30)[:, 1:29, 1:29],
                        cv[ci][:, 1:T].rearrange(
                            "p (y x) -> p y x", y=28, x=28))
                    # rowshift plane: canvas rows 1..29 at rows 0..28
                    nc.gpsimd.memset(pad[:, 864:900], 0.0)
                    nc.gpsimd.tensor_copy(pad[:, 0:870], pad[:, 930:1800])
                    # colshift plane: canvas shifted 1 col (tail stays zero)
                    nc.gpsimd.memset(pad[:, 2696:2700], 0.0)
                    nc.gpsimd.tensor_copy(pad[:, 1800:2699], pad[:, 901:1800])
                    pads.append(pad)
                return {"b": b, "par": b % 2, "cv": cv, "pads": pads,
                        "ys": {}, "qk2": {}}

            def emit_conv(ctx, i):
                """Conv for proj i (fp8 DoubleRow diag matmuls) -> y."""
                pads, cv = ctx["pads"], ctx["cv"]
                y2 = yp.tile([96, 1600], BF16, tag=f"y{i}", name=f"y{i}")
                for ci in range(2):
                    cps = psA.tile([128, 1024], F32, tag="mm", name="mm")
                    # 2-plane windows: [rowshift|canvas], [canvas|colshift]
                    vA = pads[ci][:, 0:1800].rearrange(
                        "p (j y x) -> p j y x", j=2, y=30, x=30)
                    vB = pads[ci][:, 900:2700].rearrange(
                        "p (j y x) -> p j y x", j=2, y=30, x=30)
                    # pairs 0-2 = taps (t+3, t); 3 = (6,7); 4 = (8, zero)
                    PAIR = [(vA, 0, 0), (vA, 0, 1), (vA, 0, 2),
                            (vB, 2, 0), (vB, 2, 2)]
                    for (y0, ny, po) in ROWSEG:
                        for pr, (vw, dy, dx) in enumerate(PAIR):
                            blk = (i * 5 + pr) * 192
                            nc.tensor.matmul(
                                cps[0:96, po:po + ny * 28],
                                wd_sb[ci][:, blk:blk + 192].rearrange(
                                    "p (j m) -> p j m", j=2),
                                vw[:, :, y0 + dy:y0 + dy + ny, dx:dx + 28],
                                start=(pr == 0), stop=(pr == 4),
                                perf_mode=mybir.MatmulPerfMode.DoubleRow)
                    # psum -> y with BN bias folded (per-partition)
                    bcol = bnt_sb[ci][:, i:i + 1]
                    yb = 800 * ci
                    for (y0, ny, po) in ROWSEG:
                        dst = y2[:, yb + 1 + y0 * 28:yb + 1 + (y0 + ny) * 28]
                        src = cps[0:96, po:po + ny * 28]
                        nc.vector.tensor_scalar(
                            dst, src, bcol, None, OP.add)
                    nc.vector.tensor_copy(y2[:, yb:yb + 1], cv[ci][:, 0:1])
                ctx["ys"][i] = y2

            def emit_proj(ctx, i):
                """qkv projection for proj i from y."""
                yv = ctx["ys"][i][:].rearrange("p (j t) -> p j t", j=2)
                if i < 2:
                    qrow = []
                    for ob, (o0, osz) in enumerate([(0, 128), (128, 64)]):
                        ps = psA.tile([128, 1024], F32, tag="mm", name="mm")
                        for (n0, nn) in NSEG:
                            for ci in range(2):
                                nc.tensor.matmul(
                                    ps[0:osz, n0:n0 + nn],
                                    wq_sb[i][ci][:, o0:o0 + osz],
                                    yv[:, ci, n0:n0 + nn],
                                    start=(ci == 0), stop=(ci == 1))
                        dst = qkp.tile([osz, T], BF16, tag=f"qk{i}{ob}",
                                       name=f"qk{i}{ob}")
                        nc.vector.tensor_copy(dst[:], ps[0:osz, 0:T])
                        qrow.append(dst)
                    ctx["qk2"][i] = qrow
                else:
                    # v token-major -> per-tblk vaug (3 heads fused)
                    par = ctx["par"]
                    for tb, (t0, tn) in enumerate(TBLK):
                        ps = psA.tile([128, 1024], F32, tag="mm", name="mm")
                        for ci in range(2):
                            nc.tensor.matmul(
                                ps[0:tn, 0:CO],
                                yv[:, ci, t0:t0 + tn],
                                wq_sb[2][ci][:],
                                start=(ci == 0), stop=(ci == 1))
                        dst3 = vaug[par][tb][:].rearrange(
                            "p (h d) -> p h d", h=3)[0:tn, :, 0:64]
                        eng = nc.scalar.copy if tb % 2 == 0 else \
                            nc.vector.tensor_copy
                        eng(dst3,
                            ps[0:tn, 0:CO].rearrange("p (h d) -> p h d", h=3))

            def emit_head(ctx, h):
                """Scores/exp/PV + normalize for head h of sample ctx."""
                par = ctx["par"]

                def head_rows(qk):
                    if h < 2:
                        return qk[0][h * 64:(h + 1) * 64, :]
                    return qk[1][0:64, :]
                kh = head_rows(ctx["qk2"][1])
                qh = head_rows(ctx["qk2"][0])
                vh = [vaug[par][tb][:, h * 65:(h + 1) * 65]
                      for tb in range(len(TBLK))]
                pv = psA.tile([128, 1024], F32, tag="mm", name="mm")
                es_ = [None] * len(TBLK)

                def emit_scores(tb):
                    t0, tn = TBLK[tb]
                    ss = psA.tile([128, 1024], F32, tag="mm", name="mm")
                    for (n0, nn) in NSEG:
                        nc.tensor.matmul(
                            ss[0:tn, n0:n0 + nn],
                            kh[:, t0:t0 + tn], qh[:, n0:n0 + nn],
                            start=True, stop=True)
                    e = ep.tile([128, T], BF16, tag="E", name="E")
                    nc.scalar.activation(e[0:tn, 0:T], ss[0:tn, 0:T],
                                         AF.Exp, scale=SCALE)
                    es_[tb] = e

                def emit_pv(tb):
                    t0, tn = TBLK[tb]
                    for (n0, nn) in NSEG:
                        nc.tensor.matmul(
                            pv[0:65, n0:n0 + nn],
                            vh[tb][0:tn, :],
                            es_[tb][0:tn, n0:n0 + nn],
                            start=(tb == 0), stop=(tb == len(TBLK) - 1))

                emit_scores(0)
                emit_scores(1)
                emit_scores(2)
                for tb in range(len(TBLK)):
                    if tb + 3 < len(TBLK):
                        emit_scores(tb + 3)
                    emit_pv(tb)

                den = smallp.tile([1, T], F32, tag="den", name="den")
                nc.scalar.copy(den[0:1, :], pv[64:65, 0:T])
                r = smallp.tile([1, T], F32, tag="r", name="r")
                nc.vector.reciprocal_approx_fast(r[0:1, :], den[0:1, :])
                rb = smallp.tile([64, T], F32, tag="rb", name="rb")
                nc.gpsimd.partition_broadcast(rb[:], r[0:1, :])
                dst = (aT0[par][h * 64:(h + 1) * 64, :] if h < 2
                       else aT1[par][0:64, :])
                nc.vector.tensor_tensor(dst, pv[0:64, 0:T], rb[:], OP.mult)

            def emit_final(ctx):
                """Final projection (bias via ones row) + store."""
                b, par = ctx["b"], ctx["par"]
                obuf = op_.tile([128, 6 * CO], BF16, tag="obuf", name="obuf")
                otl = op_.tile([17, CO], BF16, tag="otl", name="otl")
                for tb, (t0, tn) in enumerate(TBLK):
                    fp = psA.tile([128, 1024], F32, tag="mm", name="mm")
                    nc.tensor.matmul(fp[0:tn, 0:CO], aT0[par][:, t0:t0 + tn],
                                     wpa_sb[0][:], start=True, stop=False)
                    nc.tensor.matmul(fp[0:tn, 0:CO], aT1[par][:, t0:t0 + tn],
                                     wpa_sb[1][:], start=False, stop=True)
                    dst = obuf[:, tb * CO:tb * CO + CO] if tb < 6 else otl[:]
                    nc.vector.tensor_copy(dst[0:tn, :], fp[0:tn, 0:CO])
                nc.sync.dma_start(
                    out_d[b, 0:768, :].rearrange("(n p) c -> p n c", p=128),
                    obuf[:].rearrange("p (n c) -> p n c", n=6, c=CO))
                nc.sync.dma_start(out_d[b, 768:785, :], otl[:])

            # software pipeline: sample b's conv/qkv interleaved between
            # sample b-1's attention heads (PE stays fed; ramp stays hot)
            prev = None
            pend = None  # sample whose final is deferred past the next head0
            ctx0 = emit_load(0)
            emit_consts()
            for b in range(BPC + 1):
                cur = (ctx0 if b == 0 else emit_load(b)) if b < BPC else None
                if b == 0:
                    # no attention to interleave: delay each projection one
                    # conv stream so it is not stalled on the y copies
                    emit_conv(cur, 0)
                    emit_conv(cur, 1)
                    emit_proj(cur, 0)
                    emit_conv(cur, 2)
                    emit_proj(cur, 1)
                    emit_proj(cur, 2)
                else:
                    for i in range(3):
                        if prev is not None:
                            emit_head(prev, i)
                        if i == 0 and pend is not None:
                            emit_final(pend)
                            pend = None
                        if cur is not None:
                            emit_conv(cur, i)
                            emit_proj(cur, i)
                    pend = prev
                prev = cur
            if pend is not None:
                emit_final(pend)
    if not nc.is_finalized():
        nc.finalize()
    return nc


_NC_CACHE = None


def kernel(**inputs):
    global _NC_CACHE
    x = np.asarray(inputs["x"], dtype=np.float32)
    conv_w = np.asarray(inputs["conv_w"], dtype=np.float32)  # [3,C,1,3,3]
    bn_scale = np.asarray(inputs["bn_scale"], dtype=np.float32)
    bn_bias = np.asarray(inputs["bn_bias"], dtype=np.float32)
    bn_mean = np.asarray(inputs["bn_mean"], dtype=np.float32)
    bn_var = np.asarray(inputs["bn_var"], dtype=np.float32)
    w_qkv = np.asarray(inputs["w_qkv"], dtype=np.float32)  # [3,CO,C]
    w_proj = np.asarray(inputs["w_proj"], dtype=np.float32)  # [CO,CO]
    b_proj = np.asarray(inputs["b_proj"], dtype=np.float32)  # [CO]

    # fold BN into conv taps: y = conv(x, w)*s + (b - mu*s)
    s = bn_scale / np.sqrt(bn_var + BN_EPS)  # [3,C]
    wtap = (conv_w[:, :, 0, :, :].reshape(3, C, 9)
            * s[:, :, None]).astype(np.float32)  # [3,C,9]
    # diag conv weights, DoubleRow pairs: [2, 96, 3*5*2*96]
    # pair planes: 0-2 -> (tap t+3, tap t); 3 -> (tap6, tap7); 4 -> (tap8, 0)
    PAIR_TAPS = [(3, 0), (4, 1), (5, 2), (6, 7), (8, None)]
    wdiag_h = np.zeros((2, 96, 3, 5, 2, 96), dtype=np.float32)
    idx = np.arange(96)
    for ci, (c0, cp) in enumerate(CCH):
        for i in range(3):
            for pr, (t0_, t1_) in enumerate(PAIR_TAPS):
                wdiag_h[ci, idx, i, pr, 0, idx] = wtap[i, c0 + idx, t0_]
                if t1_ is not None:
                    wdiag_h[ci, idx, i, pr, 1, idx] = wtap[i, c0 + idx, t1_]
    wdiag_h = np.ascontiguousarray(
        wdiag_h.reshape(2, 96, 3 * 5 * 2 * 96)).astype(
        ml_dtypes.float8_e4m3)
    bnt_h = np.ascontiguousarray(
        (bn_bias - bn_mean * s).T).astype(np.float32)  # [C,3]
    wqkvT_h = np.ascontiguousarray(
        w_qkv.transpose(0, 2, 1)).astype(ml_dtypes.bfloat16)  # [3,C,CO]
    wpa_h = np.concatenate(
        [w_proj.T, b_proj[None, :]], axis=0).astype(ml_dtypes.bfloat16)

    if _NC_CACHE is None:
        _NC_CACHE = build_bass()
    nc = _NC_CACHE

    # bf16 x, token dim padded to TP=800 for the XBAR DMA transpose
    xpad = np.zeros((NCORES, BPC, TP, CP), dtype=ml_dtypes.bfloat16)
    xpad[:, :, 0:T, 0:C] = x.reshape(NCORES, BPC, T, C).astype(ml_dtypes.bfloat16)
    in_maps = [
        {"x": np.ascontiguousarray(xpad[c]), "wqkvT": wqkvT_h,
         "wdiag": wdiag_h, "bnt": bnt_h, "wpa": wpa_h}
        for c in range(NCORES)
    ]
    res = run_bass_kernel_spmd(nc, in_maps, list(range(NCORES)), **RUN_KWARGS)
    global LAST_RESULTS
    LAST_RESULTS = res
    out = np.concatenate([np.asarray(r["out"]) for r in res.results], axis=0)
    return out.reshape(B, T, CO).astype(np.float32)


RUN_KWARGS = {}
LAST_RESULTS = None


# revision 24
# speedup vs baseline: 1.1915x; 1.1915x over previous
"""Trainium2 Bass kernel for nn_Attention_51634096833229.

Conv-projection attention block (CvT-style): depthwise 3x3 conv + BN on the
28x28 token image for each of q/k/v, linear qkv projections, 3-head attention
over 785 tokens (784 image + 1 cls), output projection.

Sharding: data-parallel over batch, B=32 -> 4 samples per core on 8 cores.

Per-core design (software-pipelined: sample b's conv/qkv emitted between
sample b-1's attention heads so the PE stream stays dense):
  x bf16 (host-converted, T padded to 800, C padded to 224)
    --XBAR DMA-transpose--> xT [192,800] bf16 (2 tiles, c 0:128 / 96:192)
  dw-conv+BN on the PE as fp8 diag matmuls: per-channel tap weights on the
  diagonal, 2 taps per DoubleRow matmul via a 3-plane shifted pad canvas
  [rowshift|canvas|colshift]; PSUM rows split (16,12) to stay in banks;
  BN bias folded into the psum->sbuf y copy (per-partition scalar)
  q,k: feature-major bf16 matmul -> qT,kT [192,785]
  v:   token-major bf16 matmul -> per-tblk vaug [t,3*65] with ones cols
  scores^T [t,l] = kT_h^T qT_h bf16; exp on ACT (scale folded, no max sub);
  PV: outT_h [65,l] accumulated over t; row 64 = softmax denominators;
  scores emitted 3 t-tiles ahead of PV to cover the exp latency
  normalize: ACT-staged den + DVE reciprocal_approx_fast + gpsimd
  partition-broadcast + DVE mult
  final: token-major matmul (lhsT = aT chunks, rhs = w_proj^T with ones-row
  bias) -> out bf16 --DMA--> DRAM, host converts to f32
"""

import sys

sys.path.insert(0, "/opt/trn_rl_repo")

import numpy as np
import ml_dtypes

import concourse.bass as bass
import concourse.mybir as mybir
import concourse.tile as tile
from concourse import bacc
from concourse.bass_utils import run_bass_kernel_spmd

F32 = mybir.dt.float32
BF16 = mybir.dt.bfloat16
FP8 = mybir.dt.float8e4
AF = mybir.ActivationFunctionType
OP = mybir.AluOpType

B, T, C, CO, NH, D = 32, 785, 192, 192, 3, 64
TP = 800  # host-padded token count (multiple of 16 for DMA transpose)
CP = 224  # host-padded channel count (so chunk1 transpose lands at rows 0-95)
HH = WW = 28
NCORES = 8
BPC = B // NCORES  # samples per core
SCALE = float(CO) ** -0.5
BN_EPS = 1e-5

# token blocks of 128 along T
TBLK = [(i * 128, min(128, T - i * 128)) for i in range((T + 127) // 128)]
# channel chunks along C=192
CCH = [(0, 96), (96, 96)]
# N segments within 785 (psum bank = 512 f32)
NSEG = [(0, 512), (512, T - 512)]
# conv psum row segments: (img row0, n rows, psum col offset)
ROWSEG = [(0, 16, 0), (16, 12, 512)]


def build_bass():
    nc = bacc.Bacc(None)
    x_d = nc.declare_dram_parameter("x", [BPC, TP, CP], BF16, isOutput=False)
    wqkvT_d = nc.declare_dram_parameter("wqkvT", [3, C, CO], BF16, isOutput=False)
    wdiag_d = nc.declare_dram_parameter("wdiag", [2, 96, 3 * 5 * 2 * 96], FP8,
                                        isOutput=False)
    bnt_d = nc.declare_dram_parameter("bnt", [C, 3], F32, isOutput=False)
    wpa_d = nc.declare_dram_parameter("wpa", [C + 1, CO], BF16, isOutput=False)
    out_d = nc.declare_dram_parameter("out", [BPC, T, CO], BF16, isOutput=True)

    from contextlib import ExitStack
    with tile.TileContext(nc) as tc, ExitStack() as es:
        consts = es.enter_context(tc.tile_pool(name="consts", bufs=1))
        psA = es.enter_context(tc.tile_pool(name="psA", bufs=4, space="PSUM"))
        xTp = es.enter_context(tc.tile_pool(name="xT", bufs=2))
        padp = es.enter_context(tc.tile_pool(name="pad", bufs=2))
        yp = es.enter_context(tc.tile_pool(name="y", bufs=2))
        qkp = es.enter_context(tc.tile_pool(name="qk", bufs=2))
        q2p = es.enter_context(tc.tile_pool(name="q2", bufs=2))
        vap = es.enter_context(tc.tile_pool(name="va", bufs=2))
        ep = es.enter_context(tc.tile_pool(name="E", bufs=6))
        atp = es.enter_context(tc.tile_pool(name="aT", bufs=2))
        op_ = es.enter_context(tc.tile_pool(name="osb", bufs=3))
        smallp = es.enter_context(tc.tile_pool(name="small", bufs=3))
        if True:
            wq_sb, wd_sb, bnt_sb, wpa_sb = [], [], [], []

            def emit_consts():
                # weights into SBUF, split by channel chunk (emitted after
                # sample 0's x DMAs so the first conv isn't queued behind them)
                for i in range(3):
                    row = []
                    for ci, (c0, cp) in enumerate(CCH):
                        t = consts.tile([cp, CO], BF16, tag=f"wq{i}{ci}",
                                        name=f"wq{i}{ci}")
                        nc.sync.dma_start(t[:], wqkvT_d[i, c0:c0 + cp, :])
                        row.append(t)
                    wq_sb.append(row)
                for ci, (c0, cp) in enumerate(CCH):
                    t = consts.tile([96, 3 * 5 * 2 * 96], FP8, tag=f"wd{ci}",
                                    name=f"wd{ci}")
                    nc.sync.dma_start(t[:], wdiag_d[ci])
                    wd_sb.append(t)
                    t2 = consts.tile([cp, 3], F32, tag=f"bnt{ci}", name=f"bnt{ci}")
                    nc.sync.dma_start(t2[:], bnt_d[c0:c0 + cp, :])
                    bnt_sb.append(t2)
                wpa0 = consts.tile([128, CO], BF16, tag="wpa0", name="wpa0")
                nc.sync.dma_start(wpa0[:], wpa_d[0:128, :])
                wpa1 = consts.tile([65, CO], BF16, tag="wpa1", name="wpa1")
                nc.sync.dma_start(wpa1[:], wpa_d[128:193, :])
                wpa_sb.extend([wpa0, wpa1])

            # parity-double-buffered persistent tiles (pipeline overlap)
            vaug = [[vap.tile([128, 3 * 65], BF16, tag=f"va{p_}{tb}",
                              name=f"va{p_}{tb}") for tb in range(len(TBLK))]
                    for p_ in range(2)]
            for p_ in range(2):
                for tb, (t0, tn) in enumerate(TBLK):
                    ones3 = vaug[p_][tb][:].rearrange(
                        "p (h d) -> p h d", h=3)[0:tn, :, 64:65]
                    nc.vector.memset(ones3, 1.0)
            aT0 = [atp.tile([128, T], BF16, tag=f"aT0{p_}", name=f"aT0{p_}")
                   for p_ in range(2)]
            aT1 = [atp.tile([65, T], BF16, tag=f"aT1{p_}", name=f"aT1{p_}")
                   for p_ in range(2)]
            for p_ in range(2):
                nc.vector.memset(aT1[p_][64:65, :], 1.0)

            def emit_load(b):
                """DMA-transposed x load + fp8 3-plane pad canvases."""
                xlo = xTp.tile([128, TP], BF16, tag="xlo", name="xlo")
                xhi = xTp.tile([128, TP], BF16, tag="xhi", name="xhi")
                nc.sync.dma_start(xlo[:], x_d[b, :, 0:128], transpose=True)
                nc.sync.dma_start(xhi[:], x_d[b, :, 96:224], transpose=True)
                cv = [xlo[0:96], xhi[0:96]]
                pads = []
                for ci in range(2):
                    pad = padp.tile([96, 2700], FP8, tag=f"pad{ci}",
                                    name=f"pad{ci}")
                    nc.vector.memset(pad[:, 900:1800], 0.0)
                    nc.vector.tensor_copy(
                        pad[:, 900:1800].rearrange(
                            "p (y x) -> p y x", y=30, x=30)[:, 1:29, 1:29],
                        cv[ci][:, 1:T].rearrange(
                            "p (y x) -> p y x", y=28, x=28))
                    # rowshift plane: canvas rows 1..29 at rows 0..28
                    nc.gpsimd.memset(pad[:, 864:900], 0.0)
                    nc.gpsimd.tensor_copy(pad[:, 0:870], pad[:, 930:1800])
                    # colshift plane: canvas shifted 1 col (tail stays zero)
                    nc.gpsimd.memset(pad[:, 2696:2700], 0.0)
                    nc.gpsimd.tensor_copy(pad[:, 1800:2699], pad[:, 901:1800])
                    pads.append(pad)
                return {"b": b, "par": b % 2, "cv": cv, "pads": pads,
                        "ys": {}, "qk2": {}}

            def emit_conv(ctx, i):
                """Conv for proj i (fp8 DoubleRow diag matmuls) -> y."""
                pads, cv = ctx["pads"], ctx["cv"]
                y2 = yp.tile([96, 1600], BF16, tag=f"y{i}", name=f"y{i}")
                for ci in range(2):
                    cps = psA.tile([128, 1024], F32, tag="mm", name="mm")
                    # 2-plane windows: [rowshift|canvas], [canvas|colshift]
                    vA = pads[ci][:, 0:1800].rearrange(
                        "p (j y x) -> p j y x", j=2, y=30, x=30)
                    vB = pads[ci][:, 900:2700].rearrange(
                        "p (j y x) -> p j y x", j=2, y=30, x=30)
                    # pairs 0-2 = taps (t+3, t); 3 = (6,7); 4 = (8, zero)
                    PAIR = [(vA, 0, 0), (vA, 0, 1), (vA, 0, 2),
                            (vB, 2, 0), (vB, 2, 2)]
                    for (y0, ny, po) in ROWSEG:
                        for pr, (vw, dy, dx) in enumerate(PAIR):
                            blk = (i * 5 + pr) * 192
                            nc.tensor.matmul(
                                cps[0:96, po:po + ny * 28],
                                wd_sb[ci][:, blk:blk + 192].rearrange(
                                    "p (j m) -> p j m", j=2),
                                vw[:, :, y0 + dy:y0 + dy + ny, dx:dx + 28],
                                start=(pr == 0), stop=(pr == 4),
                                perf_mode=mybir.MatmulPerfMode.DoubleRow)
                    # psum -> y with BN bias folded (per-partition)
                    bcol = bnt_sb[ci][:, i:i + 1]
                    yb = 800 * ci
                    for (y0, ny, po) in ROWSEG:
                        dst = y2[:, yb + 1 + y0 * 28:yb + 1 + (y0 + ny) * 28]
                        src = cps[0:96, po:po + ny * 28]
                        nc.vector.tensor_scalar(
                            dst, src, bcol, None, OP.add)
                    nc.vector.tensor_copy(y2[:, yb:yb + 1], cv[ci][:, 0:1])
                ctx["ys"][i] = y2

            def emit_proj(ctx, i):
                """qkv projection for proj i from y."""
                yv = ctx["ys"][i][:].rearrange("p (j t) -> p j t", j=2)
                if i < 2:
                    qrow = []
                    for ob, (o0, osz) in enumerate([(0, 128), (128, 64)]):
                        ps = psA.tile([128, 1024], F32, tag="mm", name="mm")
                        for (n0, nn) in NSEG:
                            for ci in range(2):
                                nc.tensor.matmul(
                                    ps[0:osz, n0:n0 + nn],
                                    wq_sb[i][ci][:, o0:o0 + osz],
                                    yv[:, ci, n0:n0 + nn],
                                    start=(ci == 0), stop=(ci == 1))
                        dst = qkp.tile([osz, T], BF16, tag=f"qk{i}{ob}",
                                       name=f"qk{i}{ob}")
                        # split casts across ACT/DVE so both proj psum tiles
                        # release in parallel (head-start ss allocs wait on them)
                        if ob == 0:
                            nc.scalar.copy(dst[:], ps[0:osz, 0:T])
                        else:
                            nc.vector.tensor_copy(dst[:], ps[0:osz, 0:T])
                        qrow.append(dst)
                    ctx["qk2"][i] = qrow
                else:
                    # v token-major -> per-tblk vaug (3 heads fused)
                    par = ctx["par"]
                    for tb, (t0, tn) in enumerate(TBLK):
                        ps = psA.tile([128, 1024], F32, tag="mm", name="mm")
                        for ci in range(2):
                            nc.tensor.matmul(
                                ps[0:tn, 0:CO],
                                yv[:, ci, t0:t0 + tn],
                                wq_sb[2][ci][:],
                                start=(ci == 0), stop=(ci == 1))
                        dst3 = vaug[par][tb][:].rearrange(
                            "p (h d) -> p h d", h=3)[0:tn, :, 0:64]
                        eng = nc.scalar.copy if tb % 2 == 0 else \
                            nc.vector.tensor_copy
                        eng(dst3,
                            ps[0:tn, 0:CO].rearrange("p (h d) -> p h d", h=3))

            def emit_head(ctx, h):
                """Scores/exp/PV + normalize for head h of sample ctx."""
                par = ctx["par"]

                def head_rows(qk):
                    if h < 2:
                        return qk[0][h * 64:(h + 1) * 64, :]
                    return qk[1][0:64, :]
                kh = head_rows(ctx["qk2"][1])
                qh = head_rows(ctx["qk2"][0])
                vh = [vaug[par][tb][:, h * 65:(h + 1) * 65]
                      for tb in range(len(TBLK))]
                pv = psA.tile([128, 1024], F32, tag="mm", name="mm")
                es_ = [None] * len(TBLK)

                def emit_scores(tb):
                    t0, tn = TBLK[tb]
                    ss = psA.tile([128, 1024], F32, tag="mm", name="mm")
                    for (n0, nn) in NSEG:
                        nc.tensor.matmul(
                            ss[0:tn, n0:n0 + nn],
                            kh[:, t0:t0 + tn], qh[:, n0:n0 + nn],
                            start=True, stop=True)
                    e = ep.tile([128, T], BF16, tag="E", name="E")
                    nc.scalar.activation(e[0:tn, 0:T], ss[0:tn, 0:T],
                                         AF.Exp, scale=SCALE)
                    es_[tb] = e

                def emit_pv(tb):
                    t0, tn = TBLK[tb]
                    for (n0, nn) in NSEG:
                        nc.tensor.matmul(
                            pv[0:65, n0:n0 + nn],
                            vh[tb][0:tn, :],
                            es_[tb][0:tn, n0:n0 + nn],
                            start=(tb == 0), stop=(tb == len(TBLK) - 1))

                emit_scores(0)
                emit_scores(1)
                emit_scores(2)
                for tb in range(len(TBLK)):
                    if tb + 3 < len(TBLK):
                        emit_scores(tb + 3)
                    emit_pv(tb)

                den = smallp.tile([1, T], F32, tag="den", name="den")
                nc.scalar.copy(den[0:1, :], pv[64:65, 0:T])
                r = smallp.tile([1, T], F32, tag="r", name="r")
                nc.vector.reciprocal_approx_fast(r[0:1, :], den[0:1, :])
                rb = smallp.tile([64, T], F32, tag="rb", name="rb")
                nc.gpsimd.partition_broadcast(rb[:], r[0:1, :])
                dst = (aT0[par][h * 64:(h + 1) * 64, :] if h < 2
                       else aT1[par][0:64, :])
                nc.vector.tensor_tensor(dst, pv[0:64, 0:T], rb[:], OP.mult)

            def emit_final(ctx):
                """Final projection (bias via ones row) + store."""
                b, par = ctx["b"], ctx["par"]
                obuf = op_.tile([128, 6 * CO], BF16, tag="obuf", name="obuf")
                otl = op_.tile([17, CO], BF16, tag="otl", name="otl")
                for tb, (t0, tn) in enumerate(TBLK):
                    fp = psA.tile([128, 1024], F32, tag="mm", name="mm")
                    nc.tensor.matmul(fp[0:tn, 0:CO], aT0[par][:, t0:t0 + tn],
                                     wpa_sb[0][:], start=True, stop=False)
                    nc.tensor.matmul(fp[0:tn, 0:CO], aT1[par][:, t0:t0 + tn],
                                     wpa_sb[1][:], start=False, stop=True)
                    dst = obuf[:, tb * CO:tb * CO + CO] if tb < 6 else otl[:]
                    nc.vector.tensor_copy(dst[0:tn, :], fp[0:tn, 0:CO])
                nc.sync.dma_start(
                    out_d[b, 0:768, :].rearrange("(n p) c -> p n c", p=128),
                    obuf[:].rearrange("p (n c) -> p n c", n=6, c=CO))
                nc.sync.dma_start(out_d[b, 768:785, :], otl[:])

            # software pipeline: sample b's conv/qkv interleaved between
            # sample b-1's attention heads (PE stays fed; ramp stays hot)
            prev = None
            pend = None  # sample whose final is deferred past the next head0
            ctx0 = emit_load(0)
            emit_consts()
            for b in range(BPC + 1):
                cur = (ctx0 if b == 0 else emit_load(b)) if b < BPC else None
                if b == 0:
                    # no attention to interleave: delay each projection one
                    # conv stream so it is not stalled on the y copies
                    emit_conv(cur, 0)
                    emit_conv(cur, 1)
                    emit_proj(cur, 0)
                    emit_conv(cur, 2)
                    emit_proj(cur, 1)
                    emit_proj(cur, 2)
                else:
                    for i in range(3):
                        if prev is not None:
                            emit_head(prev, i)
                        if i == 0 and pend is not None:
                            emit_final(pend)
                            pend = None
                        if cur is not None:
                            emit_conv(cur, i)
                            emit_proj(cur, i)
                    pend = prev
                prev = cur
            if pend is not None:
                emit_final(pend)
    if not nc.is_finalized():
        nc.finalize()
    return nc


_NC_CACHE = None


def kernel(**inputs):
    global _NC_CACHE
    x = np.asarray(inputs["x"], dtype=np.float32)
    conv_w = np.asarray(inputs["conv_w"], dtype=np.float32)  # [3,C,1,3,3]
    bn_scale = np.asarray(inputs["bn_scale"], dtype=np.float32)
    bn_bias = np.asarray(inputs["bn_bias"], dtype=np.float32)
    bn_mean = np.asarray(inputs["bn_mean"], dtype=np.float32)
    bn_var = np.asarray(inputs["bn_var"], dtype=np.float32)
    w_qkv = np.asarray(inputs["w_qkv"], dtype=np.float32)  # [3,CO,C]
    w_proj = np.asarray(inputs["w_proj"], dtype=np.float32)  # [CO,CO]
    b_proj = np.asarray(inputs["b_proj"], dtype=np.float32)  # [CO]

    # fold BN into conv taps: y = conv(x, w)*s + (b - mu*s)
    s = bn_scale / np.sqrt(bn_var + BN_EPS)  # [3,C]
    wtap = (conv_w[:, :, 0, :, :].reshape(3, C, 9)
            * s[:, :, None]).astype(np.float32)  # [3,C,9]
    # diag conv weights, DoubleRow pairs: [2, 96, 3*5*2*96]
    # pair planes: 0-2 -> (tap t+3, tap t); 3 -> (tap6, tap7); 4 -> (tap8, 0)
    PAIR_TAPS = [(3, 0), (4, 1), (5, 2), (6, 7), (8, None)]
    wdiag_h = np.zeros((2, 96, 3, 5, 2, 96), dtype=np.float32)
    idx = np.arange(96)
    for ci, (c0, cp) in enumerate(CCH):
        for i in range(3):
            for pr, (t0_, t1_) in enumerate(PAIR_TAPS):
                wdiag_h[ci, idx, i, pr, 0, idx] = wtap[i, c0 + idx, t0_]
                if t1_ is not None:
                    wdiag_h[ci, idx, i, pr, 1, idx] = wtap[i, c0 + idx, t1_]
    wdiag_h = np.ascontiguousarray(
        wdiag_h.reshape(2, 96, 3 * 5 * 2 * 96)).astype(
        ml_dtypes.float8_e4m3)
    bnt_h = np.ascontiguousarray(
        (bn_bias - bn_mean * s).T).astype(np.float32)  # [C,3]
    wqkvT_h = np.ascontiguousarray(
        w_qkv.transpose(0, 2, 1)).astype(ml_dtypes.bfloat16)  # [3,C,CO]
    wpa_h = np.concatenate(
        [w_proj.T, b_proj[None, :]], axis=0).astype(ml_dtypes.bfloat16)

    if _NC_CACHE is None:
        _NC_CACHE = build_bass()
    nc = _NC_CACHE

    # bf16 x, token dim padded to TP=800 for the XBAR DMA transpose
    xpad = np.zeros((NCORES, BPC, TP, CP), dtype=ml_dtypes.bfloat16)
    xpad[:, :, 0:T, 0:C] = x.reshape(NCORES, BPC, T, C).astype(ml_dtypes.bfloat16)
    in_maps = [
        {"x": np.ascontiguousarray(xpad[c]), "wqkvT": wqkvT_h,
         "wdiag": wdiag_h, "bnt": bnt_h, "wpa": wpa_h}
        for c in range(NCORES)
    ]
    res = run_bass_kernel_spmd(nc, in_maps, list(range(NCORES)), **RUN_KWARGS)
    global LAST_RESULTS
    LAST_RESULTS = res
    out = np.concatenate([np.asarray(r["out"]) for r in res.results], axis=0)
    return out.reshape(B, T, CO).astype(np.float32)


RUN_KWARGS = {}
LAST_RESULTS = None


# revision 25
# speedup vs baseline: 1.2039x; 1.0104x over previous
"""Trainium2 Bass kernel for nn_Attention_51634096833229.

Conv-projection attention block (CvT-style): depthwise 3x3 conv + BN on the
28x28 token image for each of q/k/v, linear qkv projections, 3-head attention
over 785 tokens (784 image + 1 cls), output projection.

Sharding: data-parallel over batch, B=32 -> 4 samples per core on 8 cores.

Per-core design (software-pipelined: sample b's conv/qkv emitted between
sample b-1's attention heads so the PE stream stays dense):
  x bf16 (host-converted, T padded to 800, C padded to 224)
    --XBAR DMA-transpose--> xT [192,800] bf16 (2 tiles, c 0:128 / 96:192)
  dw-conv+BN on the PE as fp8 diag matmuls: per-channel tap weights on the
  diagonal, 2 taps per DoubleRow matmul via a 3-plane shifted pad canvas
  [rowshift|canvas|colshift]; PSUM rows split (16,12) to stay in banks;
  BN bias folded into the psum->sbuf y copy (per-partition scalar)
  q,k: feature-major bf16 matmul -> qT,kT [192,785]
  v:   token-major bf16 matmul -> per-tblk vaug [t,3*65] with ones cols
  scores^T [t,l] = kT_h^T qT_h bf16; exp on ACT (scale folded, no max sub);
  PV: outT_h [65,l] accumulated over t; row 64 = softmax denominators;
  scores emitted 3 t-tiles ahead of PV to cover the exp latency
  normalize: ACT-staged den + DVE reciprocal_approx_fast + gpsimd
  partition-broadcast + DVE mult
  final: token-major matmul (lhsT = aT chunks, rhs = w_proj^T with ones-row
  bias) -> out bf16 --DMA--> DRAM, host converts to f32
"""

import sys

sys.path.insert(0, "/opt/trn_rl_repo")

import numpy as np
import ml_dtypes

import concourse.bass as bass
import concourse.mybir as mybir
import concourse.tile as tile
from concourse import bacc
from concourse.bass_utils import run_bass_kernel_spmd

F32 = mybir.dt.float32
BF16 = mybir.dt.bfloat16
FP8 = mybir.dt.float8e4
AF = mybir.ActivationFunctionType
OP = mybir.AluOpType

B, T, C, CO, NH, D = 32, 785, 192, 192, 3, 64
TP = 800  # host-padded token count (multiple of 16 for DMA transpose)
CP = 224  # host-padded channel count (so chunk1 transpose lands at rows 0-95)
HH = WW = 28
NCORES = 8
BPC = B // NCORES  # samples per core
SCALE = float(CO) ** -0.5
BN_EPS = 1e-5

# token blocks of 128 along T
TBLK = [(i * 128, min(128, T - i * 128)) for i in range((T + 127) // 128)]
# channel chunks along C=192
CCH = [(0, 96), (96, 96)]
# N segments within 785 (psum bank = 512 f32)
NSEG = [(0, 512), (512, T - 512)]
# conv psum row segments: (img row0, n rows, psum col offset)
ROWSEG = [(0, 14, 0), (14, 14, 512)]


def build_bass():
    nc = bacc.Bacc(None)
    x_d = nc.declare_dram_parameter("x", [BPC, TP, CP], BF16, isOutput=False)
    wqkvT_d = nc.declare_dram_parameter("wqkvT", [3, C, CO], BF16, isOutput=False)
    wdiag_d = nc.declare_dram_parameter("wdiag", [2, 96, 3 * 5 * 2 * 96], FP8,
                                        isOutput=False)
    bnt_d = nc.declare_dram_parameter("bnt", [C, 3], F32, isOutput=False)
    wpa_d = nc.declare_dram_parameter("wpa", [C + 1, CO], BF16, isOutput=False)
    out_d = nc.declare_dram_parameter("out", [BPC, T, CO], BF16, isOutput=True)

    from contextlib import ExitStack
    with tile.TileContext(nc) as tc, ExitStack() as es:
        consts = es.enter_context(tc.tile_pool(name="consts", bufs=1))
        psA = es.enter_context(tc.tile_pool(name="psA", bufs=4, space="PSUM"))
        xTp = es.enter_context(tc.tile_pool(name="xT", bufs=2))
        padp = es.enter_context(tc.tile_pool(name="pad", bufs=2))
        yp = es.enter_context(tc.tile_pool(name="y", bufs=2))
        qkp = es.enter_context(tc.tile_pool(name="qk", bufs=2))
        q2p = es.enter_context(tc.tile_pool(name="q2", bufs=2))
        vap = es.enter_context(tc.tile_pool(name="va", bufs=2))
        ep = es.enter_context(tc.tile_pool(name="E", bufs=6))
        atp = es.enter_context(tc.tile_pool(name="aT", bufs=2))
        op_ = es.enter_context(tc.tile_pool(name="osb", bufs=3))
        smallp = es.enter_context(tc.tile_pool(name="small", bufs=3))
        if True:
            wq_sb, wd_sb, bnt_sb, wpa_sb = [], [], [], []

            def emit_consts():
                # weights into SBUF, split by channel chunk (emitted after
                # sample 0's x DMAs so the first conv isn't queued behind them)
                for i in range(3):
                    row = []
                    for ci, (c0, cp) in enumerate(CCH):
                        t = consts.tile([cp, CO], BF16, tag=f"wq{i}{ci}",
                                        name=f"wq{i}{ci}")
                        nc.sync.dma_start(t[:], wqkvT_d[i, c0:c0 + cp, :])
                        row.append(t)
                    wq_sb.append(row)
                for ci, (c0, cp) in enumerate(CCH):
                    t = consts.tile([96, 3 * 5 * 2 * 96], FP8, tag=f"wd{ci}",
                                    name=f"wd{ci}")
                    nc.sync.dma_start(t[:], wdiag_d[ci])
                    wd_sb.append(t)
                    t2 = consts.tile([cp, 3], F32, tag=f"bnt{ci}", name=f"bnt{ci}")
                    nc.sync.dma_start(t2[:], bnt_d[c0:c0 + cp, :])
                    bnt_sb.append(t2)
                wpa0 = consts.tile([128, CO], BF16, tag="wpa0", name="wpa0")
                nc.sync.dma_start(wpa0[:], wpa_d[0:128, :])
                wpa1 = consts.tile([65, CO], BF16, tag="wpa1", name="wpa1")
                nc.sync.dma_start(wpa1[:], wpa_d[128:193, :])
                wpa_sb.extend([wpa0, wpa1])

            # parity-double-buffered persistent tiles (pipeline overlap)
            vaug = [[vap.tile([128, 3 * 65], BF16, tag=f"va{p_}{tb}",
                              name=f"va{p_}{tb}") for tb in range(len(TBLK))]
                    for p_ in range(2)]
            for p_ in range(2):
                for tb, (t0, tn) in enumerate(TBLK):
                    ones3 = vaug[p_][tb][:].rearrange(
                        "p (h d) -> p h d", h=3)[0:tn, :, 64:65]
                    nc.vector.memset(ones3, 1.0)
            aT0 = [atp.tile([128, T], BF16, tag=f"aT0{p_}", name=f"aT0{p_}")
                   for p_ in range(2)]
            aT1 = [atp.tile([65, T], BF16, tag=f"aT1{p_}", name=f"aT1{p_}")
                   for p_ in range(2)]
            for p_ in range(2):
                nc.vector.memset(aT1[p_][64:65, :], 1.0)

            def emit_load(b):
                """DMA-transposed x load + fp8 3-plane pad canvases."""
                xlo = xTp.tile([128, TP], BF16, tag="xlo", name="xlo")
                xhi = xTp.tile([128, TP], BF16, tag="xhi", name="xhi")
                nc.sync.dma_start(xlo[:], x_d[b, :, 0:128], transpose=True)
                nc.sync.dma_start(xhi[:], x_d[b, :, 96:224], transpose=True)
                cv = [xlo[0:96], xhi[0:96]]
                pads = []
                for ci in range(2):
                    pad = padp.tile([96, 2700], FP8, tag=f"pad{ci}",
                                    name=f"pad{ci}")
                    nc.vector.memset(pad[:, 900:1800], 0.0)
                    nc.vector.tensor_copy(
                        pad[:, 900:1800].rearrange(
                            "p (y x) -> p y x", y=30, x=30)[:, 1:29, 1:29],
                        cv[ci][:, 1:T].rearrange(
                            "p (y x) -> p y x", y=28, x=28))
                    # rowshift plane: canvas rows 1..29 at rows 0..28
                    nc.gpsimd.memset(pad[:, 864:900], 0.0)
                    nc.gpsimd.tensor_copy(pad[:, 0:870], pad[:, 930:1800])
                    # colshift plane: canvas shifted 1 col (tail stays zero)
                    nc.gpsimd.memset(pad[:, 2696:2700], 0.0)
                    nc.gpsimd.tensor_copy(pad[:, 1800:2699], pad[:, 901:1800])
                    pads.append(pad)
                return {"b": b, "par": b % 2, "cv": cv, "pads": pads,
                        "ys": {}, "qk2": {}}

            def emit_conv(ctx, i):
                """Conv for proj i (fp8 DoubleRow diag matmuls) -> y."""
                pads, cv = ctx["pads"], ctx["cv"]
                y2 = yp.tile([96, 1600], BF16, tag=f"y{i}", name=f"y{i}")
                for ci in range(2):
                    cps = psA.tile([128, 1024], F32, tag="mm", name="mm")
                    # 2-plane windows: [rowshift|canvas], [canvas|colshift]
                    vA = pads[ci][:, 0:1800].rearrange(
                        "p (j y x) -> p j y x", j=2, y=30, x=30)
                    vB = pads[ci][:, 900:2700].rearrange(
                        "p (j y x) -> p j y x", j=2, y=30, x=30)
                    # pairs 0-2 = taps (t+3, t); 3 = (6,7); 4 = (8, zero)
                    PAIR = [(vA, 0, 0), (vA, 0, 1), (vA, 0, 2),
                            (vB, 2, 0), (vB, 2, 2)]
                    for (y0, ny, po) in ROWSEG:
                        for pr, (vw, dy, dx) in enumerate(PAIR):
                            blk = (i * 5 + pr) * 192
                            nc.tensor.matmul(
                                cps[0:96, po:po + ny * 28],
                                wd_sb[ci][:, blk:blk + 192].rearrange(
                                    "p (j m) -> p j m", j=2),
                                vw[:, :, y0 + dy:y0 + dy + ny, dx:dx + 28],
                                start=(pr == 0), stop=(pr == 4),
                                perf_mode=mybir.MatmulPerfMode.DoubleRow)
                    # psum -> y with BN bias folded (per-partition)
                    bcol = bnt_sb[ci][:, i:i + 1]
                    yb = 800 * ci
                    for (y0, ny, po) in ROWSEG:
                        dst = y2[:, yb + 1 + y0 * 28:yb + 1 + (y0 + ny) * 28]
                        src = cps[0:96, po:po + ny * 28]
                        nc.vector.tensor_scalar(
                            dst, src, bcol, None, OP.add)
                    nc.vector.tensor_copy(y2[:, yb:yb + 1], cv[ci][:, 0:1])
                ctx["ys"][i] = y2

            def emit_proj(ctx, i):
                """qkv projection for proj i from y."""
                yv = ctx["ys"][i][:].rearrange("p (j t) -> p j t", j=2)
                if i < 2:
                    qrow = []
                    for ob, (o0, osz) in enumerate([(0, 128), (128, 64)]):
                        ps = psA.tile([128, 1024], F32, tag="mm", name="mm")
                        for (n0, nn) in NSEG:
                            for ci in range(2):
                                nc.tensor.matmul(
                                    ps[0:osz, n0:n0 + nn],
                                    wq_sb[i][ci][:, o0:o0 + osz],
                                    yv[:, ci, n0:n0 + nn],
                                    start=(ci == 0), stop=(ci == 1))
                        dst = qkp.tile([osz, T], BF16, tag=f"qk{i}{ob}",
                                       name=f"qk{i}{ob}")
                        # split casts across ACT/DVE so both proj psum tiles
                        # release in parallel (head-start ss allocs wait on them)
                        if ob == 0:
                            nc.scalar.copy(dst[:], ps[0:osz, 0:T])
                        else:
                            nc.vector.tensor_copy(dst[:], ps[0:osz, 0:T])
                        qrow.append(dst)
                    ctx["qk2"][i] = qrow
                else:
                    # v token-major -> per-tblk vaug (3 heads fused)
                    par = ctx["par"]
                    for tb, (t0, tn) in enumerate(TBLK):
                        ps = psA.tile([128, 1024], F32, tag="mm", name="mm")
                        for ci in range(2):
                            nc.tensor.matmul(
                                ps[0:tn, 0:CO],
                                yv[:, ci, t0:t0 + tn],
                                wq_sb[2][ci][:],
                                start=(ci == 0), stop=(ci == 1))
                        dst3 = vaug[par][tb][:].rearrange(
                            "p (h d) -> p h d", h=3)[0:tn, :, 0:64]
                        eng = nc.scalar.copy if tb % 2 == 0 else \
                            nc.vector.tensor_copy
                        eng(dst3,
                            ps[0:tn, 0:CO].rearrange("p (h d) -> p h d", h=3))

            def emit_head(ctx, h):
                """Scores/exp/PV + normalize for head h of sample ctx."""
                par = ctx["par"]

                def head_rows(qk):
                    if h < 2:
                        return qk[0][h * 64:(h + 1) * 64, :]
                    return qk[1][0:64, :]
                kh = head_rows(ctx["qk2"][1])
                qh = head_rows(ctx["qk2"][0])
                vh = [vaug[par][tb][:, h * 65:(h + 1) * 65]
                      for tb in range(len(TBLK))]
                pv = psA.tile([128, 1024], F32, tag="mm", name="mm")
                es_ = [None] * len(TBLK)

                def emit_scores(tb):
                    t0, tn = TBLK[tb]
                    ss = psA.tile([128, 1024], F32, tag="mm", name="mm")
                    for (n0, nn) in NSEG:
                        nc.tensor.matmul(
                            ss[0:tn, n0:n0 + nn],
                            kh[:, t0:t0 + tn], qh[:, n0:n0 + nn],
                            start=True, stop=True)
                    e = ep.tile([128, T], BF16, tag="E", name="E")
                    nc.scalar.activation(e[0:tn, 0:T], ss[0:tn, 0:T],
                                         AF.Exp, scale=SCALE)
                    es_[tb] = e

                def emit_pv(tb):
                    t0, tn = TBLK[tb]
                    for (n0, nn) in NSEG:
                        nc.tensor.matmul(
                            pv[0:65, n0:n0 + nn],
                            vh[tb][0:tn, :],
                            es_[tb][0:tn, n0:n0 + nn],
                            start=(tb == 0), stop=(tb == len(TBLK) - 1))

                emit_scores(0)
                emit_scores(1)
                emit_scores(2)
                for tb in range(len(TBLK)):
                    if tb + 3 < len(TBLK):
                        emit_scores(tb + 3)
                    emit_pv(tb)

                den = smallp.tile([1, T], F32, tag="den", name="den")
                nc.scalar.copy(den[0:1, :], pv[64:65, 0:T])
                r = smallp.tile([1, T], F32, tag="r", name="r")
                nc.vector.reciprocal_approx_fast(r[0:1, :], den[0:1, :])
                rb = smallp.tile([64, T], F32, tag="rb", name="rb")
                nc.gpsimd.partition_broadcast(rb[:], r[0:1, :])
                dst = (aT0[par][h * 64:(h + 1) * 64, :] if h < 2
                       else aT1[par][0:64, :])
                nc.vector.tensor_tensor(dst, pv[0:64, 0:T], rb[:], OP.mult)

            def emit_final(ctx):
                """Final projection (bias via ones row) + store."""
                b, par = ctx["b"], ctx["par"]
                obuf = op_.tile([128, 6 * CO], BF16, tag="obuf", name="obuf")
                otl = op_.tile([17, CO], BF16, tag="otl", name="otl")
                for tb, (t0, tn) in enumerate(TBLK):
                    fp = psA.tile([128, 1024], F32, tag="mm", name="mm")
                    nc.tensor.matmul(fp[0:tn, 0:CO], aT0[par][:, t0:t0 + tn],
                                     wpa_sb[0][:], start=True, stop=False)
                    nc.tensor.matmul(fp[0:tn, 0:CO], aT1[par][:, t0:t0 + tn],
                                     wpa_sb[1][:], start=False, stop=True)
                    dst = obuf[:, tb * CO:tb * CO + CO] if tb < 6 else otl[:]
                    nc.vector.tensor_copy(dst[0:tn, :], fp[0:tn, 0:CO])
                nc.sync.dma_start(
                    out_d[b, 0:768, :].rearrange("(n p) c -> p n c", p=128),
                    obuf[:].rearrange("p (n c) -> p n c", n=6, c=CO))
                nc.sync.dma_start(out_d[b, 768:785, :], otl[:])

            # software pipeline: sample b's conv/qkv interleaved between
            # sample b-1's attention heads (PE stays fed; ramp stays hot)
            prev = None
            pend = None  # sample whose final is deferred past the next head0
            ctx0 = emit_load(0)
            emit_consts()
            for b in range(BPC + 1):
                cur = (ctx0 if b == 0 else emit_load(b)) if b < BPC else None
                if b == 0:
                    # no attention to interleave: delay each projection one
                    # conv stream so it is not stalled on the y copies
                    emit_conv(cur, 0)
                    emit_conv(cur, 1)
                    emit_proj(cur, 0)
                    emit_conv(cur, 2)
                    emit_proj(cur, 1)
                    emit_proj(cur, 2)
                else:
                    for i in range(3):
                        if prev is not None:
                            emit_head(prev, i)
                        if i == 0 and pend is not None:
                            emit_final(pend)
                            pend = None
                        if cur is not None:
                            emit_conv(cur, i)
                            emit_proj(cur, i)
                    pend = prev
                prev = cur
            if pend is not None:
                emit_final(pend)
    if not nc.is_finalized():
        nc.finalize()
    return nc


_NC_CACHE = None


def kernel(**inputs):
    global _NC_CACHE
    x = np.asarray(inputs["x"], dtype=np.float32)
    conv_w = np.asarray(inputs["conv_w"], dtype=np.float32)  # [3,C,1,3,3]
    bn_scale = np.asarray(inputs["bn_scale"], dtype=np.float32)
    bn_bias = np.asarray(inputs["bn_bias"], dtype=np.float32)
    bn_mean = np.asarray(inputs["bn_mean"], dtype=np.float32)
    bn_var = np.asarray(inputs["bn_var"], dtype=np.float32)
    w_qkv = np.asarray(inputs["w_qkv"], dtype=np.float32)  # [3,CO,C]
    w_proj = np.asarray(inputs["w_proj"], dtype=np.float32)  # [CO,CO]
    b_proj = np.asarray(inputs["b_proj"], dtype=np.float32)  # [CO]

    # fold BN into conv taps: y = conv(x, w)*s + (b - mu*s)
    s = bn_scale / np.sqrt(bn_var + BN_EPS)  # [3,C]
    wtap = (conv_w[:, :, 0, :, :].reshape(3, C, 9)
            * s[:, :, None]).astype(np.float32)  # [3,C,9]
    # diag conv weights, DoubleRow pairs: [2, 96, 3*5*2*96]
    # pair planes: 0-2 -> (tap t+3, tap t); 3 -> (tap6, tap7); 4 -> (tap8, 0)
    PAIR_TAPS = [(3, 0), (4, 1), (5, 2), (6, 7), (8, None)]
    wdiag_h = np.zeros((2, 96, 3, 5, 2, 96), dtype=np.float32)
    idx = np.arange(96)
    for ci, (c0, cp) in enumerate(CCH):
        for i in range(3):
            for pr, (t0_, t1_) in enumerate(PAIR_TAPS):
                wdiag_h[ci, idx, i, pr, 0, idx] = wtap[i, c0 + idx, t0_]
                if t1_ is not None:
                    wdiag_h[ci, idx, i, pr, 1, idx] = wtap[i, c0 + idx, t1_]
    wdiag_h = np.ascontiguousarray(
        wdiag_h.reshape(2, 96, 3 * 5 * 2 * 96)).astype(
        ml_dtypes.float8_e4m3)
    bnt_h = np.ascontiguousarray(
        (bn_bias - bn_mean * s).T).astype(np.float32)  # [C,3]
    wqkvT_h = np.ascontiguousarray(
        w_qkv.transpose(0, 2, 1)).astype(ml_dtypes.bfloat16)  # [3,C,CO]
    wpa_h = np.concatenate(
        [w_proj.T, b_proj[None, :]], axis=0).astype(ml_dtypes.bfloat16)

    if _NC_CACHE is None:
        _NC_CACHE = build_bass()
    nc = _NC_CACHE

    # bf16 x, token dim padded to TP=800 for the XBAR DMA transpose
    xpad = np.zeros((NCORES, BPC, TP, CP), dtype=ml_dtypes.bfloat16)
    xpad[:, :, 0:T, 0:C] = x.reshape(NCORES, BPC, T, C).astype(ml_dtypes.bfloat16)
    in_maps = [
        {"x": np.ascontiguousarray(xpad[c]), "wqkvT": wqkvT_h,
         "wdiag": wdiag_h, "bnt": bnt_h, "wpa": wpa_h}
        for c in range(NCORES)
    ]
    res = run_bass_kernel_spmd(nc, in_maps, list(range(NCORES)), **RUN_KWARGS)
    global LAST_RESULTS
    LAST_RESULTS = res
    out = np.concatenate([np.asarray(r["out"]) for r in res.results], axis=0)
    return out.reshape(B, T, CO).astype(np.float32)


RUN_KWARGS = {}
LAST_RESULTS = None
